# revision 10
# baseline (speedup 1.0000x reference)
"""Trainium2 Bass kernel for 2-layer GAT (nn_GAT_53386443489918).

Strategy (8 NeuronCores, SPMD):
- Shard nodes by dst range: 1250 nodes/core, padded to 1280 = 10 blocks x 128.
- Host-side index-only prep: per core, balance its nodes into 10 blocks by
  degree, group edges by block, pad each block's edge list to T_B tiles of 128.
- Phase 0: per-core  h = x_shard @ W1  (+ per-node attention stats), AllGather
  node table hx = [h | a_src]  ->  every core holds the full table.
- Phase 1 (edge pass, layer 1): per 128-edge tile, indirect-DMA gather of
  hx[src], one-hot Sel matrix via is_equal vs iota, per-edge softmax numerator
  ex = exp(leakyrelu(a_src+a_dst)); aggregate  Sel.T @ [ex*h | ex]  into PSUM
  accumulated over the block's tiles; divide by the summed denominator after
  aggregation (segment softmax without materializing alpha).
- Phase 2: h2 = relu(out1) @ W2 per core + stats, AllGather small table.
- Phase 3: same edge pass for layer 2 (CLS=40, 1 head), write output shard.
"""
import sys

for _p in ("/opt/trn_rl_repo",):
    if _p not in sys.path:
        sys.path.insert(0, _p)

import numpy as np

import concourse.bacc as bacc
import concourse.bass as bass
import concourse.mybir as mybir
import concourse.tile as tile
from concourse.bass import IndirectOffsetOnAxis
from concourse.bass_utils import run_bass_kernel_spmd

F32 = mybir.dt.float32
F32R = mybir.dt.float32r
BF16 = mybir.dt.bfloat16
I32 = mybir.dt.int32
Alu = mybir.AluOpType
Act = mybir.ActivationFunctionType

N = 10000
E = 160000
F_IN = 128
HID = 128
HEADS = 16
D1 = HEADS * HID            # 2048
CLS = 40
NCORES = 8
NPC = N // NCORES           # 1250
NBLK = 10
P = 128
NPC_PAD = NBLK * P          # 1280
NROWS = NCORES * NPC_PAD    # 10240
HXW = D1 + HEADS            # 2064 (h | a_src)
H2W = CLS + 1               # 41   (h2 | a_src2)
PAD_DST = 300.0

_cache = {}
_last_results = None


# --------------------------------------------------------------------------
# host-side index-only preprocessing
# --------------------------------------------------------------------------
def prep_indices(edge_index):
    src = np.concatenate([edge_index[0], np.arange(N, dtype=np.int64)])
    dst = np.concatenate([edge_index[1], np.arange(N, dtype=np.int64)])

    slot_of = np.full(N, -1, dtype=np.int64)
    cores = []
    for c in range(NCORES):
        lo, hi = c * NPC, (c + 1) * NPC
        emask = (dst >= lo) & (dst < hi)
        e_src = src[emask]
        e_dst_loc = dst[emask] - lo
        deg = np.bincount(e_dst_loc, minlength=NPC)
        order = np.argsort(-deg, kind="stable")
        blk_load = np.zeros(NBLK, dtype=np.int64)
        blk_fill = np.zeros(NBLK, dtype=np.int64)
        node_blk = np.empty(NPC, dtype=np.int64)
        node_slot = np.empty(NPC, dtype=np.int64)
        for v in order:
            cand = np.where(blk_fill < P)[0]
            b = cand[np.argmin(blk_load[cand])]
            node_blk[v] = b
            node_slot[v] = blk_fill[b]
            blk_fill[b] += 1
            blk_load[b] += deg[v]
        slot_of[lo:hi] = node_blk * P + node_slot
        edge_blk = node_blk[e_dst_loc]
        edge_slot = node_slot[e_dst_loc]
        blocks = []
        empty = np.ones((NBLK, P), dtype=bool)
        empty[edge_blk, edge_slot] = False
        for b in range(NBLK):
            bm = edge_blk == b
            blocks.append((e_src[bm], edge_slot[bm]))
        cores.append((blocks, empty, slot_of[lo:hi].copy(), lo))

    maxcnt = 0
    for blocks, empty, _, _ in cores:
        for b in range(NBLK):
            maxcnt = max(maxcnt, len(blocks[b][0]) + int(empty[b].sum()))
    T_B = int(np.ceil(maxcnt / P))

    row_of = (np.arange(N) // NPC) * NPC_PAD + slot_of  # global padded row

    per_core = []
    for blocks, empty, slot, lo in cores:
        src_idx = np.zeros((NBLK, T_B * P), dtype=np.int32)
        dst_loc = np.full((NBLK, T_B * P), PAD_DST, dtype=np.float32)
        for b in range(NBLK):
            es, esl = blocks[b]
            dums = np.where(empty[b])[0]
            es = np.concatenate([row_of[es], row_of[np.zeros(len(dums), dtype=np.int64)]])
            esl = np.concatenate([esl, dums])
            n = len(es)
            src_idx[b, :n] = es
            dst_loc[b, :n] = esl.astype(np.float32)
        # SBUF layout: [128, NBLK*T_B]; column (b*T_B+t) = tile t of block b
        si = src_idx.reshape(NBLK, T_B, P).transpose(2, 0, 1).reshape(P, NBLK * T_B)
        dl = dst_loc.reshape(NBLK, T_B, P).transpose(2, 0, 1).reshape(P, NBLK * T_B)
        per_core.append((np.ascontiguousarray(si), np.ascontiguousarray(dl), slot, lo))
    return per_core, T_B


# --------------------------------------------------------------------------
# device program
# --------------------------------------------------------------------------
def build_program(T_B):
    import contextlib

    nc = bacc.Bacc(
        "TRN2",
        target_bir_lowering=False,
        debug=False,
        enable_asserts=False,
        num_devices=NCORES,
    )
    NT = NBLK * T_B

    xT = nc.dram_tensor("xT", [P, NPC_PAD], F32, kind="ExternalInput").ap()
    W1 = nc.dram_tensor("W1", [P, D1], F32, kind="ExternalInput").ap()
    att1s = nc.dram_tensor("att1s", [P, D1], F32, kind="ExternalInput").ap()
    att1d = nc.dram_tensor("att1d", [P, D1], F32, kind="ExternalInput").ap()
    b1r = nc.dram_tensor("b1r", [P, D1], F32, kind="ExternalInput").ap()
    W2r = nc.dram_tensor("W2r", [P, 16 * CLS], F32, kind="ExternalInput").ap()
    att2s = nc.dram_tensor("att2s", [P, CLS], F32, kind="ExternalInput").ap()
    att2d = nc.dram_tensor("att2d", [P, CLS], F32, kind="ExternalInput").ap()
    b2r = nc.dram_tensor("b2r", [P, CLS], F32, kind="ExternalInput").ap()
    srcidx = nc.dram_tensor("srcidx", [P, NT], I32, kind="ExternalInput").ap()
    dstloc = nc.dram_tensor("dstloc", [P, NT], F32, kind="ExternalInput").ap()
    iota = nc.dram_tensor("iota", [P, P], F32, kind="ExternalInput").ap()
    ident = nc.dram_tensor("ident", [P, P], F32, kind="ExternalInput").ap()
    out_ext = nc.dram_tensor("out", [NPC_PAD, CLS], F32, kind="ExternalOutput").ap()

    hx_loc = nc.dram_tensor("hx_loc", [NPC_PAD, HXW], BF16).ap()
    hx_full = nc.dram_tensor("hx_full", [NROWS, HXW], BF16, addr_space="Shared").ap()
    h2x_loc = nc.dram_tensor("h2x_loc", [NPC_PAD, H2W], F32).ap()
    h2x_full = nc.dram_tensor("h2x_full", [NROWS, H2W], F32, addr_space="Shared").ap()

    rg = [list(range(NCORES))]

    with tile.TileContext(nc) as tc:
        with contextlib.ExitStack() as top:
            persist = top.enter_context(tc.tile_pool(name="persist", bufs=1))
            reluT = persist.tile([P, NBLK * 16 * P], BF16, tag="reluT")
            adst1 = persist.tile([P, NBLK * HEADS], BF16, tag="adst1")
            adst2 = persist.tile([P, NBLK], F32, tag="adst2")
            si_sb = persist.tile([P, NT], I32, tag="si")
            dl_sb = persist.tile([P, NT], F32, tag="dl")
            iota_sb = persist.tile([P, P], F32, tag="iota")
            ident_sb = persist.tile([P, P], F32, tag="ident")
            identb_sb = persist.tile([P, P], BF16, tag="identb")
            nc.sync.dma_start(out=si_sb[:], in_=srcidx)
            nc.sync.dma_start(out=dl_sb[:], in_=dstloc)
            nc.sync.dma_start(out=iota_sb[:], in_=iota)
            nc.sync.dma_start(out=ident_sb[:], in_=ident)
            nc.vector.tensor_copy(out=identb_sb[:], in_=ident_sb[:])

            # ---------------- phase 0: h = x @ W1, stats, AllGather ----------
            with contextlib.ExitStack() as ph:
                const = ph.enter_context(tc.tile_pool(name="p0c", bufs=1))
                sb = ph.enter_context(tc.tile_pool(name="p0s", bufs=2))
                ps = ph.enter_context(tc.tile_pool(name="p0p", bufs=2, space="PSUM"))
                xT_sb = const.tile([P, NPC_PAD], F32, tag="xT")
                W1_sb = const.tile([P, D1], F32, tag="W1")
                a1s_sb = const.tile([P, D1], F32, tag="a1s")
                a1d_sb = const.tile([P, D1], F32, tag="a1d")
                nc.sync.dma_start(out=xT_sb[:], in_=xT)
                nc.sync.dma_start(out=W1_sb[:], in_=W1)
                nc.sync.dma_start(out=a1s_sb[:], in_=att1s)
                nc.sync.dma_start(out=a1d_sb[:], in_=att1d)
                for m in range(NBLK):
                    hp = ps.tile([P, D1], F32, tag="hp")
                    for j in range(4):
                        nc.tensor.matmul(
                            out=hp[:, j * 512:(j + 1) * 512],
                            lhsT=xT_sb[:, m * P:(m + 1) * P],
                            rhs=W1_sb[:, j * 512:(j + 1) * 512],
                            start=True, stop=True,
                        )
                    hx_sb = sb.tile([P, HXW], BF16, tag="hx")
                    tmp = sb.tile([P, D1], F32, tag="tmp")
                    stat = sb.tile([P, HEADS], F32, tag="stat")
                    nc.vector.tensor_copy(out=hx_sb[:, :D1], in_=hp[:])
                    nc.vector.tensor_tensor(out=tmp[:], in0=hp[:], in1=a1s_sb[:], op=Alu.mult)
                    nc.vector.reduce_sum(
                        out=stat[:],
                        in_=tmp[:].rearrange("p (h c) -> p h c", h=HEADS),
                        axis=mybir.AxisListType.X,
                    )
                    nc.vector.tensor_copy(out=hx_sb[:, D1:HXW], in_=stat[:])
                    nc.vector.tensor_tensor(out=tmp[:], in0=hp[:], in1=a1d_sb[:], op=Alu.mult)
                    nc.vector.reduce_sum(
                        out=stat[:],
                        in_=tmp[:].rearrange("p (h c) -> p h c", h=HEADS),
                        axis=mybir.AxisListType.X,
                    )
                    nc.vector.tensor_copy(
                        out=adst1[:, m * HEADS:(m + 1) * HEADS], in_=stat[:])
                    nc.sync.dma_start(out=hx_loc[m * P:(m + 1) * P, :], in_=hx_sb[:])
            nc.gpsimd.collective_compute(
                "AllGather", Alu.bypass, replica_groups=rg,
                ins=[hx_loc], outs=[hx_full],
            )

            # ---------------- phase 1: layer-1 edge pass ---------------------
            with contextlib.ExitStack() as ph:
                const = ph.enter_context(tc.tile_pool(name="p1c", bufs=1))
                gp = ph.enter_context(tc.tile_pool(name="p1g", bufs=3))
                mp = ph.enter_context(tc.tile_pool(name="p1m", bufs=2))
                sp = ph.enter_context(tc.tile_pool(name="p1s", bufs=3))
                op = ph.enter_context(tc.tile_pool(name="p1o", bufs=2))
                bigp = ph.enter_context(tc.tile_pool(name="p1bp", bufs=1, space="PSUM"))
                scrp = ph.enter_context(tc.tile_pool(name="p1sp", bufs=2, space="PSUM"))
                adp = ph.enter_context(tc.tile_pool(name="p1ap", bufs=1, space="PSUM"))
                b1_sb = const.tile([P, D1], F32, tag="b1r")
                nc.sync.dma_start(out=b1_sb[:], in_=b1r)
                for b in range(NBLK):
                    acc = bigp.tile([P, HXW], F32, tag="acc")
                    for t in range(T_B):
                        col = b * T_B + t
                        g = gp.tile([P, HXW], BF16, tag="g")
                        nc.gpsimd.indirect_dma_start(
                            out=g[:], out_offset=None,
                            in_=hx_full,
                            in_offset=IndirectOffsetOnAxis(ap=si_sb[:, col:col + 1], axis=0),
                        )
                        sel = sp.tile([P, P], BF16, tag="sel")
                        nc.vector.tensor_tensor(
                            out=sel[:],
                            in0=dl_sb[:, col:col + 1].to_broadcast([P, P]),
                            in1=iota_sb[:], op=Alu.is_equal,
                        )
                        selT_ps = scrp.tile([P, P], BF16, tag="scrb")
                        nc.tensor.transpose(out=selT_ps[:], in_=sel[:], identity=identb_sb[:])
                        selT = sp.tile([P, P], BF16, tag="selT")
                        nc.scalar.copy(out=selT[:], in_=selT_ps[:])
                        adpe = adp.tile([P, HEADS], F32, tag="adpe")
                        nc.tensor.matmul(
                            out=adpe[:], lhsT=selT[:],
                            rhs=adst1[:, b * HEADS:(b + 1) * HEADS],
                            start=True, stop=True,
                        )
                        ex = sp.tile([P, HEADS], F32, tag="ex")
                        lr = sp.tile([P, HEADS], F32, tag="lr")
                        nc.vector.tensor_tensor(
                            out=lr[:], in0=g[:, D1:HXW], in1=adpe[:], op=Alu.add)
                        nc.vector.tensor_scalar_mul(out=ex[:], in0=lr[:], scalar1=0.2)
                        nc.vector.tensor_tensor(out=lr[:], in0=lr[:], in1=ex[:], op=Alu.max)
                        nc.scalar.activation(out=ex[:], in_=lr[:], func=Act.Exp)
                        M = mp.tile([P, HXW], BF16, tag="M")
                        nc.vector.tensor_tensor(
                            out=M[:, :D1].rearrange("p (h c) -> p h c", h=HEADS),
                            in0=g[:, :D1].rearrange("p (h c) -> p h c", h=HEADS),
                            in1=ex[:][:, :, None].to_broadcast([P, HEADS, HID]),
                            op=Alu.mult,
                        )
                        nc.vector.tensor_copy(out=M[:, D1:HXW], in_=ex[:])
                        for j in range(4):
                            nc.tensor.matmul(
                                out=acc[:, j * 512:(j + 1) * 512],
                                lhsT=sel[:], rhs=M[:, j * 512:(j + 1) * 512],
                                start=(t == 0), stop=(t == T_B - 1),
                            )
                        nc.tensor.matmul(
                            out=acc[:, D1:HXW],
                            lhsT=sel[:], rhs=M[:, D1:HXW],
                            start=(t == 0), stop=(t == T_B - 1),
                        )
                    # block epilogue: divide, bias, relu, transpose into reluT
                    den = sp.tile([P, HEADS], F32, tag="den")
                    nc.vector.tensor_scalar_add(out=den[:], in0=acc[:, D1:HXW], scalar1=1e-16)
                    rec = sp.tile([P, HEADS], F32, tag="rec")
                    nc.vector.reciprocal(out=rec[:], in_=den[:])
                    o1 = op.tile([P, D1], BF16, tag="o1")
                    nc.vector.tensor_tensor(
                        out=o1[:].rearrange("p (h c) -> p h c", h=HEADS),
                        in0=acc[:, :D1].rearrange("p (h c) -> p h c", h=HEADS),
                        in1=rec[:][:, :, None].to_broadcast([P, HEADS, HID]),
                        op=Alu.mult,
                    )
                    nc.vector.tensor_tensor(out=o1[:], in0=o1[:], in1=b1_sb[:], op=Alu.add)
                    nc.vector.tensor_scalar_max(out=o1[:], in0=o1[:], scalar1=0.0)
                    for k in range(16):
                        tp = scrp.tile([P, P], BF16, tag="scrb")
                        nc.tensor.transpose(
                            out=tp[:], in_=o1[:, k * P:(k + 1) * P], identity=identb_sb[:])
                        nc.scalar.copy(
                            out=reluT[:, (b * 16 + k) * P:(b * 16 + k + 1) * P], in_=tp[:])

            # ---------------- phase 2: h2 = relu1 @ W2, stats, AllGather -----
            with contextlib.ExitStack() as ph:
                const = ph.enter_context(tc.tile_pool(name="p2c", bufs=1))
                sb = ph.enter_context(tc.tile_pool(name="p2s", bufs=2))
                ps = ph.enter_context(tc.tile_pool(name="p2p", bufs=2, space="PSUM"))
                W2_sb = const.tile([P, 16 * CLS], BF16, tag="W2")
                a2s_sb = const.tile([P, CLS], F32, tag="a2s")
                a2d_sb = const.tile([P, CLS], F32, tag="a2d")
                nc.gpsimd.dma_start(out=W2_sb[:], in_=W2r)
                nc.sync.dma_start(out=a2s_sb[:], in_=att2s)
                nc.sync.dma_start(out=a2d_sb[:], in_=att2d)
                for m in range(NBLK):
                    h2p = ps.tile([P, CLS], F32, tag="h2p")
                    for k in range(16):
                        nc.tensor.matmul(
                            out=h2p[:],
                            lhsT=reluT[:, (m * 16 + k) * P:(m * 16 + k + 1) * P],
                            rhs=W2_sb[:, k * CLS:(k + 1) * CLS],
                            start=(k == 0), stop=(k == 15),
                        )
                    h2x_sb = sb.tile([P, H2W], F32, tag="h2x")
                    tmp = sb.tile([P, CLS], F32, tag="tmp2")
                    nc.vector.tensor_copy(out=h2x_sb[:, :CLS], in_=h2p[:])
                    nc.vector.tensor_tensor(out=tmp[:], in0=h2p[:], in1=a2s_sb[:], op=Alu.mult)
                    nc.vector.reduce_sum(
                        out=h2x_sb[:, CLS:H2W], in_=tmp[:], axis=mybir.AxisListType.X)
                    nc.vector.tensor_tensor(out=tmp[:], in0=h2p[:], in1=a2d_sb[:], op=Alu.mult)
                    nc.vector.reduce_sum(
                        out=adst2[:, m:m + 1], in_=tmp[:], axis=mybir.AxisListType.X)
                    nc.sync.dma_start(out=h2x_loc[m * P:(m + 1) * P, :], in_=h2x_sb[:])
            nc.gpsimd.collective_compute(
                "AllGather", Alu.bypass, replica_groups=rg,
                ins=[h2x_loc], outs=[h2x_full],
            )

            # ---------------- phase 3: layer-2 edge pass ---------------------
            with contextlib.ExitStack() as ph:
                const = ph.enter_context(tc.tile_pool(name="p3c", bufs=1))
                gp = ph.enter_context(tc.tile_pool(name="p3g", bufs=4))
                sp = ph.enter_context(tc.tile_pool(name="p3s", bufs=3))
                op = ph.enter_context(tc.tile_pool(name="p3o", bufs=2))
                rp = ph.enter_context(tc.tile_pool(name="p3rp", bufs=2, space="PSUM"))
                scrp = ph.enter_context(tc.tile_pool(name="p3sp", bufs=2, space="PSUM"))
                adp = ph.enter_context(tc.tile_pool(name="p3ap", bufs=2, space="PSUM"))
                b2_sb = const.tile([P, CLS], F32, tag="b2r")
                nc.sync.dma_start(out=b2_sb[:], in_=b2r)
                for b in range(NBLK):
                    acc = rp.tile([P, H2W], F32, tag="acc2")
                    for t in range(T_B):
                        col = b * T_B + t
                        g = gp.tile([P, H2W], F32, tag="g2")
                        nc.gpsimd.indirect_dma_start(
                            out=g[:], out_offset=None,
                            in_=h2x_full,
                            in_offset=IndirectOffsetOnAxis(ap=si_sb[:, col:col + 1], axis=0),
                        )
                        sel = sp.tile([P, P], F32, tag="sel2")
                        nc.vector.tensor_tensor(
                            out=sel[:],
                            in0=dl_sb[:, col:col + 1].to_broadcast([P, P]),
                            in1=iota_sb[:], op=Alu.is_equal,
                        )
                        selT_ps = scrp.tile([P, P], F32, tag="scr2")
                        nc.tensor.transpose(out=selT_ps[:], in_=sel[:], identity=ident_sb[:])
                        selT = sp.tile([P, P], F32, tag="selT2")
                        nc.scalar.copy(out=selT[:], in_=selT_ps[:])
                        adpe = adp.tile([P, 1], F32, tag="adpe2")
                        nc.tensor.matmul(
                            out=adpe[:], lhsT=selT[:], rhs=adst2[:, b:b + 1],
                            start=True, stop=True,
                        )
                        ex = sp.tile([P, 1], F32, tag="ex2")
                        lr = sp.tile([P, 1], F32, tag="lr2")
                        nc.vector.tensor_tensor(
                            out=lr[:], in0=g[:, CLS:H2W], in1=adpe[:], op=Alu.add)
                        nc.vector.tensor_scalar_mul(out=ex[:], in0=lr[:], scalar1=0.2)
                        nc.vector.tensor_tensor(out=lr[:], in0=lr[:], in1=ex[:], op=Alu.max)
                        nc.scalar.activation(out=ex[:], in_=lr[:], func=Act.Exp)
                        M = sp.tile([P, H2W], F32, tag="M2")
                        nc.vector.tensor_tensor(
                            out=M[:, :CLS], in0=g[:, :CLS],
                            in1=ex[:].to_broadcast([P, CLS]), op=Alu.mult,
                        )
                        nc.vector.tensor_copy(out=M[:, CLS:H2W], in_=ex[:])
                        nc.tensor.matmul(
                            out=acc[:], lhsT=sel[:], rhs=M[:],
                            start=(t == 0), stop=(t == T_B - 1),
                        )
                    den = sp.tile([P, 1], F32, tag="den2")
                    nc.vector.tensor_scalar_add(out=den[:], in0=acc[:, CLS:H2W], scalar1=1e-16)
                    rec = sp.tile([P, 1], F32, tag="rec2")
                    nc.vector.reciprocal(out=rec[:], in_=den[:])
                    o2 = op.tile([P, CLS], F32, tag="o2")
                    nc.vector.tensor_tensor(
                        out=o2[:], in0=acc[:, :CLS],
                        in1=rec[:].to_broadcast([P, CLS]), op=Alu.mult,
                    )
                    nc.vector.tensor_tensor(out=o2[:], in0=o2[:], in1=b2_sb[:], op=Alu.add)
                    nc.sync.dma_start(out=out_ext[b * P:(b + 1) * P, :], in_=o2[:])

    nc.compile()
    return nc


# --------------------------------------------------------------------------
# host wrapper
# --------------------------------------------------------------------------
def kernel(x, edge_index, W1, att_src1, att_dst1, b1, W2, att_src2, att_dst2, b2,
           trace=False):
    global _last_results
    x = np.ascontiguousarray(np.asarray(x, dtype=np.float32))
    edge_index = np.asarray(edge_index, dtype=np.int64)
    W1 = np.asarray(W1, dtype=np.float32)
    att_src1 = np.asarray(att_src1, dtype=np.float32)
    att_dst1 = np.asarray(att_dst1, dtype=np.float32)
    b1 = np.asarray(b1, dtype=np.float32)
    W2 = np.asarray(W2, dtype=np.float32)
    att_src2 = np.asarray(att_src2, dtype=np.float32)
    att_dst2 = np.asarray(att_dst2, dtype=np.float32)
    b2 = np.asarray(b2, dtype=np.float32)

    per_core, T_B = prep_indices(edge_index)

    if T_B not in _cache:
        _cache[T_B] = build_program(T_B)
    nc = _cache[T_B]

    att1s_rep = np.tile(att_src1.reshape(1, D1), (P, 1)).astype(np.float32)
    att1d_rep = np.tile(att_dst1.reshape(1, D1), (P, 1)).astype(np.float32)
    b1_rep = np.tile(b1.reshape(1, D1), (P, 1)).astype(np.float32)
    W2_re = np.ascontiguousarray(
        W2.reshape(16, P, CLS).transpose(1, 0, 2).reshape(P, 16 * CLS)).astype(np.float32)
    att2s_rep = np.tile(att_src2.reshape(1, CLS), (P, 1)).astype(np.float32)
    att2d_rep = np.tile(att_dst2.reshape(1, CLS), (P, 1)).astype(np.float32)
    b2_rep = np.tile(b2.reshape(1, CLS), (P, 1)).astype(np.float32)
    iota_row = np.tile(np.arange(P, dtype=np.float32).reshape(1, P), (P, 1))
    ident = np.eye(P, dtype=np.float32)

    in_maps = []
    for c in range(NCORES):
        si, dl, slot, lo = per_core[c]
        x_pad = np.zeros((NPC_PAD, F_IN), dtype=np.float32)
        x_pad[slot] = x[lo:lo + NPC]
        in_maps.append({
            "xT": np.ascontiguousarray(x_pad.T),
            "W1": W1, "att1s": att1s_rep, "att1d": att1d_rep, "b1r": b1_rep,
            "W2r": W2_re, "att2s": att2s_rep, "att2d": att2d_rep, "b2r": b2_rep,
            "srcidx": si, "dstloc": dl, "iota": iota_row, "ident": ident,
        })

    res = run_bass_kernel_spmd(nc, in_maps, list(range(NCORES)), trace=trace)
    _last_results = res

    out = np.empty((N, CLS), dtype=np.float32)
    for c in range(NCORES):
        si, dl, slot, lo = per_core[c]
        out[lo:lo + NPC] = res.results[c]["out"][slot]
    return out
